# revision 6
# baseline (speedup 1.0000x reference)
"""AdaptiveSpecAugment on 8 Trainium2 NeuronCores.

out[b,t,f] = 0 if (time_mask[b,t] or freq_mask[b,f]) else input_spec[b,t,f]

The masks come from jax threefry RNG with the fixed key 42, so the random
draws are replicated bit-exactly on host (tiny arrays); the device kernel
does the memory-bound streaming multiply
    out = in * keep_t[b,t] * keep_f[b,f]
with keep vectors in {0,1}.

Sharding: batch data-parallel, 32 samples per core. Per core, samples are
processed in pairs: 2*1600 = 3200 t-rows = 25 blocks of 128 partitions, so
tiles are exactly [128, 25*80] f32 with no padding. keep_t is fed as a
[128, 25] per-partition/block scalar table (broadcast along f via a
stride-0 AP dim); keep_f as [128, 80] tiles (broadcast along the block dim).
"""

import os
import sys

import numpy as np

if "/opt/trn_rl_repo" not in sys.path and not any(
    p.endswith("trn_rl_repo") for p in sys.path
):
    sys.path.insert(0, "/opt/trn_rl_repo")

B, T, F = 256, 1600, 80
N_CORES = 8
BC = B // N_CORES          # samples per core = 32
PAIRS = BC // 2            # 16
G = 2 * T // 128           # 25 blocks of 128 rows per pair
AUXW = G + 2 * F           # 25 + 160 = 185

FREQ_MASKS = 2
TIME_MASKS = 0.05
FREQ_WIDTH = 27
TIME_WIDTH = 0.05
MAX_TIME_MASKS = 10


def _keep_vectors(length: np.ndarray):
    """Replicate the reference's _spec_masks RNG bit-exactly on host CPU.

    Returns keep_t [B,T] float32 and keep_f [B,F] float32 (1.0 = keep).
    """
    import jax
    import jax.numpy as jnp

    cpu = jax.local_devices(backend="cpu")[0]
    with jax.default_device(cpu):
        length = jnp.asarray(np.asarray(length))
        key = jax.random.key(42)
        kf1, kf2, kt1, kt2 = jax.random.split(key, 4)

        x_left = jax.random.randint(kf1, (B, FREQ_MASKS), 0, F - FREQ_WIDTH + 1)
        w_f = jax.random.randint(kf2, (B, FREQ_MASKS), 0, FREQ_WIDTH + 1)
        f_idx = jnp.arange(F)
        freq_mask = jnp.any(
            (f_idx >= x_left[..., None]) & (f_idx < (x_left + w_f)[..., None]),
            axis=1,
        )  # [B, F]

        lf = length.astype(jnp.float32)
        n_t = jnp.minimum(MAX_TIME_MASKS, (lf * TIME_MASKS).astype(jnp.int32))
        tw = jnp.maximum(1, (lf * TIME_WIDTH).astype(jnp.int32))
        hi = jnp.maximum(1, length - tw)

        u1 = jax.random.uniform(kt1, (B, MAX_TIME_MASKS))
        y_left = jnp.minimum(
            (u1 * (hi[:, None] + 1).astype(jnp.float32)).astype(jnp.int32),
            hi[:, None],
        )
        u2 = jax.random.uniform(kt2, (B, MAX_TIME_MASKS))
        w_t = jnp.minimum(
            (u2 * (tw[:, None] + 1).astype(jnp.float32)).astype(jnp.int32),
            tw[:, None],
        )
        active = jnp.arange(MAX_TIME_MASKS)[None, :] < n_t[:, None]
        t_idx = jnp.arange(T)
        time_mask = jnp.any(
            active[..., None]
            & (t_idx >= y_left[..., None])
            & (t_idx < (y_left + w_t)[..., None]),
            axis=1,
        )  # [B, T]

        keep_t = 1.0 - np.asarray(time_mask, dtype=np.float32)
        keep_f = 1.0 - np.asarray(freq_mask, dtype=np.float32)
    return keep_t, keep_f


_NC_CACHE = {}


def _build_bass():
    import concourse.bass as bass
    import concourse.tile as tile
    from concourse import bacc, mybir

    f32 = mybir.dt.float32
    nc = bacc.Bacc()
    x = nc.declare_dram_parameter("x", [BC, T, F], f32, isOutput=False)
    aux = nc.declare_dram_parameter("aux", [128, PAIRS * AUXW], f32, isOutput=False)
    y = nc.declare_dram_parameter("y", [BC, T, F], f32, isOutput=True)

    # [pair, block, partition(row-in-block), f] view of the contiguous shard
    xv = x[:].flatten_outer_dims().rearrange("(pr g p) f -> pr g p f", g=G, p=128)
    yv = y[:].flatten_outer_dims().rearrange("(pr g p) f -> pr g p f", g=G, p=128)

    # pair-row = g*128 + p; sample 0 of the pair owns rows < 1600:
    # blocks 0..11 fully, block 12 partitions < 64. Sample 1 owns the rest.
    GS = T // 128  # 12 full blocks per sample
    PS = T - GS * 128  # 64: partition split inside the shared block

    with tile.TileContext(nc) as tc:
        with (
            tc.tile_pool(name="data", bufs=3) as data,
            tc.tile_pool(name="auxp", bufs=1) as auxp,
        ):
            aux_all = auxp.tile([128, PAIRS * AUXW], f32)
            nc.sync.dma_start(out=aux_all[:], in_=aux[:])
            for pr in range(PAIRS):
                atile = aux_all[:, pr * AUXW : (pr + 1) * AUXW]

                dtile = data.tile([128, G * F], f32)
                d3 = dtile[:].rearrange("p (g f) -> p g f", f=F)
                nc.sync.dma_start(out=d3, in_=xv[pr].rearrange("g p f -> p g f"))

                kt = atile[:, 0:G].unsqueeze(2).broadcast_to([128, G, F])
                nc.vector.tensor_mul(d3, d3, kt)

                kf0 = atile[:, G : G + F]
                kf1 = atile[:, G + F : G + 2 * F]
                nc.vector.tensor_mul(
                    d3[:, 0:GS, :],
                    d3[:, 0:GS, :],
                    kf0.unsqueeze(1).broadcast_to([128, GS, F]),
                )
                nc.vector.tensor_mul(
                    d3[0:PS, GS : GS + 1, :],
                    d3[0:PS, GS : GS + 1, :],
                    kf0[0:PS].unsqueeze(1).broadcast_to([PS, 1, F]),
                )
                nc.vector.tensor_mul(
                    d3[PS:128, GS : GS + 1, :],
                    d3[PS:128, GS : GS + 1, :],
                    kf1[PS:128].unsqueeze(1).broadcast_to([128 - PS, 1, F]),
                )
                nc.vector.tensor_mul(
                    d3[:, GS + 1 : G, :],
                    d3[:, GS + 1 : G, :],
                    kf1.unsqueeze(1).broadcast_to([128, G - GS - 1, F]),
                )

                nc.scalar.dma_start(out=yv[pr].rearrange("g p f -> p g f"), in_=d3)
    if not nc.is_finalized():
        nc.finalize()
    return nc


def _get_nc():
    if "nc" not in _NC_CACHE:
        _NC_CACHE["nc"] = _build_bass()
    return _NC_CACHE["nc"]


def _pack_aux(keep_t: np.ndarray, keep_f: np.ndarray) -> np.ndarray:
    # kt table: [core, pair, p, g] with value keep_t at pair-row g*128+p
    kt = keep_t.reshape(N_CORES, PAIRS, G, 128).transpose(0, 1, 3, 2)
    kf = keep_f.reshape(N_CORES, PAIRS, 2, F)
    aux = np.empty((N_CORES, PAIRS, 128, AUXW), np.float32)
    aux[..., :G] = kt
    aux[..., G : G + F] = kf[:, :, 0][:, :, None, :]
    aux[..., G + F :] = kf[:, :, 1][:, :, None, :]
    # device layout: one [128, PAIRS*AUXW] block per core
    return np.ascontiguousarray(aux.transpose(0, 2, 1, 3)).reshape(
        N_CORES, 128, PAIRS * AUXW
    )


LAST_RESULTS = None


def kernel(input_spec: np.ndarray, length: np.ndarray):
    global LAST_RESULTS
    from concourse.bass_utils import run_bass_kernel_spmd

    input_spec = np.asarray(input_spec, dtype=np.float32)
    length = np.asarray(length, dtype=np.int32)
    assert input_spec.shape == (B, T, F), input_spec.shape

    keep_t, keep_f = _keep_vectors(length)
    aux = _pack_aux(keep_t, keep_f)

    in_maps = [
        {
            "x": np.ascontiguousarray(input_spec[c * BC : (c + 1) * BC]),
            "aux": aux[c],
        }
        for c in range(N_CORES)
    ]

    nc = _get_nc()
    res = run_bass_kernel_spmd(nc, in_maps, core_ids=list(range(N_CORES)))
    LAST_RESULTS = res
    out = np.concatenate([res.results[c]["y"] for c in range(N_CORES)], axis=0)
    return out, length


# revision 8
# speedup vs baseline: 1.1018x; 1.1018x over previous
"""AdaptiveSpecAugment on 8 Trainium2 NeuronCores.

out[b,t,f] = 0 if (time_mask[b,t] or freq_mask[b,f]) else input_spec[b,t,f]

The masks come from jax threefry RNG with the fixed key 42, so the random
draws are replicated bit-exactly on host (tiny arrays); the device kernel
does the memory-bound streaming multiply
    out = in * keep_t[b,t] * keep_f[b,f]
with keep vectors in {0,1}.

Sharding: batch data-parallel, 32 samples per core. Per core, samples are
processed in groups of 4: 4*1600 = 6400 t-rows = 25 blocks of 256 rows
(2 consecutive rows per partition, so DMA descriptors move 640B contiguous
runs). keep_t is fed as a [128, 50] per-(partition, block, j) scalar table
broadcast along f via a stride-0 AP dim; keep_f as [128, 80] tiles
broadcast along blocks, with sample boundaries handled by partition-sliced
ops.
"""

import os
import sys

import numpy as np

if "/opt/trn_rl_repo" not in sys.path and not any(
    p.endswith("trn_rl_repo") for p in sys.path
):
    sys.path.insert(0, "/opt/trn_rl_repo")

B, T, F = 256, 1600, 80
N_CORES = 8
BC = B // N_CORES          # samples per core = 32
S = 4                      # samples per group
NG = BC // S               # 8 groups per core
RPB = 256                  # rows per block
RJ = 2                     # rows per partition (j dim)
G = S * T // RPB           # 25 blocks per group
AUXW = G * RJ + S * F      # 50 + 320 = 370 floats per group
ROWS = S * T               # 6400 rows per group

FREQ_MASKS = 2
TIME_MASKS = 0.05
FREQ_WIDTH = 27
TIME_WIDTH = 0.05
MAX_TIME_MASKS = 10


def _keep_vectors(length: np.ndarray):
    """Replicate the reference's _spec_masks RNG bit-exactly on host CPU.

    Returns keep_t [B,T] float32 and keep_f [B,F] float32 (1.0 = keep).
    """
    import jax
    import jax.numpy as jnp

    cpu = jax.local_devices(backend="cpu")[0]
    with jax.default_device(cpu):
        length = jnp.asarray(np.asarray(length))
        key = jax.random.key(42)
        kf1, kf2, kt1, kt2 = jax.random.split(key, 4)

        x_left = jax.random.randint(kf1, (B, FREQ_MASKS), 0, F - FREQ_WIDTH + 1)
        w_f = jax.random.randint(kf2, (B, FREQ_MASKS), 0, FREQ_WIDTH + 1)
        f_idx = jnp.arange(F)
        freq_mask = jnp.any(
            (f_idx >= x_left[..., None]) & (f_idx < (x_left + w_f)[..., None]),
            axis=1,
        )  # [B, F]

        lf = length.astype(jnp.float32)
        n_t = jnp.minimum(MAX_TIME_MASKS, (lf * TIME_MASKS).astype(jnp.int32))
        tw = jnp.maximum(1, (lf * TIME_WIDTH).astype(jnp.int32))
        hi = jnp.maximum(1, length - tw)

        u1 = jax.random.uniform(kt1, (B, MAX_TIME_MASKS))
        y_left = jnp.minimum(
            (u1 * (hi[:, None] + 1).astype(jnp.float32)).astype(jnp.int32),
            hi[:, None],
        )
        u2 = jax.random.uniform(kt2, (B, MAX_TIME_MASKS))
        w_t = jnp.minimum(
            (u2 * (tw[:, None] + 1).astype(jnp.float32)).astype(jnp.int32),
            tw[:, None],
        )
        active = jnp.arange(MAX_TIME_MASKS)[None, :] < n_t[:, None]
        t_idx = jnp.arange(T)
        time_mask = jnp.any(
            active[..., None]
            & (t_idx >= y_left[..., None])
            & (t_idx < (y_left + w_t)[..., None]),
            axis=1,
        )  # [B, T]

        keep_t = 1.0 - np.asarray(time_mask, dtype=np.float32)
        keep_f = 1.0 - np.asarray(freq_mask, dtype=np.float32)
    return keep_t, keep_f


def _sample_rects():
    """Per-sample (block-range, partition-range) rectangles within a group.

    Returns a list of (s, ops) where ops is a list of
    (g_lo, g_hi, p_lo, p_hi) rectangles covering sample s's rows.
    """
    out = []
    for s in range(S):
        rs, re = s * T, (s + 1) * T
        bs, be = rs // RPB, (re - 1) // RPB
        ops = []
        first_full = bs
        if rs % RPB:
            ps = (rs % RPB) // RJ
            ops.append((bs, bs + 1, ps, 128))
            first_full = bs + 1
        last_full_excl = be if (re % RPB) else be + 1
        if last_full_excl > first_full:
            ops.append((first_full, last_full_excl, 0, 128))
        if re % RPB:
            pe = (re % RPB) // RJ
            ops.append((be, be + 1, 0, pe))
        # HW: an AP's partition span is limited by its base partition
        # (base 0 -> 128, 32 -> 32, 64 -> 64, 96 -> 32). Split rects.
        legal = []
        for g_lo, g_hi, p_lo, p_hi in ops:
            while p_lo < p_hi:
                span = p_hi - p_lo if p_lo == 0 else min(64 if p_lo == 64 else 32, p_hi - p_lo)
                legal.append((g_lo, g_hi, p_lo, p_lo + span))
                p_lo += span
        out.append((s, legal))
    return out


_NC_CACHE = {}


def _build_bass():
    import concourse.bass as bass
    import concourse.tile as tile
    from concourse import bacc, mybir

    f32 = mybir.dt.float32
    nc = bacc.Bacc()
    x = nc.declare_dram_parameter("x", [BC, T, F], f32, isOutput=False)
    aux = nc.declare_dram_parameter("aux", [128, NG * AUXW], f32, isOutput=False)
    y = nc.declare_dram_parameter("y", [BC, T, F], f32, isOutput=True)

    # [group, block, partition, j, f] view of the contiguous shard
    xv = x[:].flatten_outer_dims().rearrange(
        "(grp g p j) f -> grp g p j f", g=G, p=128, j=RJ
    )
    yv = y[:].flatten_outer_dims().rearrange(
        "(grp g p j) f -> grp g p j f", g=G, p=128, j=RJ
    )
    rects = _sample_rects()

    with tile.TileContext(nc) as tc:
        with (
            tc.tile_pool(name="data", bufs=3) as data,
            tc.tile_pool(name="auxp", bufs=1) as auxp,
        ):
            aux_all = auxp.tile([128, NG * AUXW], f32)
            nc.sync.dma_start(out=aux_all[:], in_=aux[:])
            for grp in range(NG):
                atile = aux_all[:, grp * AUXW : (grp + 1) * AUXW]

                dtile = data.tile([128, G * RJ * F], f32)
                d4 = dtile[:].rearrange("p (g j f) -> p g j f", j=RJ, f=F)
                nc.sync.dma_start(out=d4, in_=xv[grp].rearrange("g p j f -> p g j f"))

                kt = (
                    atile[:, 0 : G * RJ]
                    .rearrange("p (g j) -> p g j", j=RJ)
                    .unsqueeze(3)
                    .broadcast_to([128, G, RJ, F])
                )
                nc.vector.tensor_mul(d4, d4, kt)

                for s, ops in rects:
                    kf = atile[:, G * RJ + s * F : G * RJ + (s + 1) * F]
                    for g_lo, g_hi, p_lo, p_hi in ops:
                        dst = d4[p_lo:p_hi, g_lo:g_hi, :, :]
                        src = (
                            kf[p_lo:p_hi]
                            .unsqueeze(1)
                            .unsqueeze(2)
                            .broadcast_to([p_hi - p_lo, g_hi - g_lo, RJ, F])
                        )
                        nc.vector.tensor_mul(dst, dst, src)

                nc.scalar.dma_start(
                    out=yv[grp].rearrange("g p j f -> p g j f"), in_=d4
                )
    if not nc.is_finalized():
        nc.finalize()
    return nc


def _get_nc():
    if "nc" not in _NC_CACHE:
        _NC_CACHE["nc"] = _build_bass()
    return _NC_CACHE["nc"]


def _pack_aux(keep_t: np.ndarray, keep_f: np.ndarray) -> np.ndarray:
    # kt table per group: [p, g*RJ + j] = keep_t at group row g*RPB + p*RJ + j
    kt = keep_t.reshape(N_CORES, NG, G, 128, RJ).transpose(0, 1, 3, 2, 4)
    kt = kt.reshape(N_CORES, NG, 128, G * RJ)
    kf = keep_f.reshape(N_CORES, NG, S, F)
    aux = np.empty((N_CORES, NG, 128, AUXW), np.float32)
    aux[..., : G * RJ] = kt
    for s in range(S):
        aux[..., G * RJ + s * F : G * RJ + (s + 1) * F] = kf[:, :, s][:, :, None, :]
    # device layout: one [128, NG*AUXW] block per core
    return np.ascontiguousarray(aux.transpose(0, 2, 1, 3)).reshape(
        N_CORES, 128, NG * AUXW
    )


LAST_RESULTS = None


def kernel(input_spec: np.ndarray, length: np.ndarray):
    global LAST_RESULTS
    from concourse.bass_utils import run_bass_kernel_spmd

    input_spec = np.asarray(input_spec, dtype=np.float32)
    length = np.asarray(length, dtype=np.int32)
    assert input_spec.shape == (B, T, F), input_spec.shape

    keep_t, keep_f = _keep_vectors(length)
    aux = _pack_aux(keep_t, keep_f)

    in_maps = [
        {
            "x": np.ascontiguousarray(input_spec[c * BC : (c + 1) * BC]),
            "aux": aux[c],
        }
        for c in range(N_CORES)
    ]

    nc = _get_nc()
    res = run_bass_kernel_spmd(nc, in_maps, core_ids=list(range(N_CORES)))
    LAST_RESULTS = res
    out = np.concatenate([res.results[c]["y"] for c in range(N_CORES)], axis=0)
    return out, length


# revision 9
# speedup vs baseline: 1.3476x; 1.2231x over previous
"""AdaptiveSpecAugment on 8 Trainium2 NeuronCores.

out[b,t,f] = 0 if (time_mask[b,t] or freq_mask[b,f]) else input_spec[b,t,f]

The masks come from jax threefry RNG with the fixed key 42, so the random
draws are replicated bit-exactly on host (tiny arrays); the device kernel
does the memory-bound streaming multiply
    out = in * keep_t[b,t] * keep_f[b,f]
with keep vectors in {0,1}.

Sharding: batch data-parallel, 32 samples per core. Per core, samples are
processed in groups of 4: 4*1600 = 6400 t-rows = 25 blocks of 256 rows
(2 consecutive rows per partition, so DMA descriptors move 640B contiguous
runs). keep_t is fed as a [128, 50] per-(partition, block, j) scalar table
broadcast along f via a stride-0 AP dim; keep_f as [128, 80] tiles
broadcast along blocks, with sample boundaries handled by partition-sliced
ops.
"""

import os
import sys

import numpy as np

if "/opt/trn_rl_repo" not in sys.path and not any(
    p.endswith("trn_rl_repo") for p in sys.path
):
    sys.path.insert(0, "/opt/trn_rl_repo")

B, T, F = 256, 1600, 80
N_CORES = 8
BC = B // N_CORES          # samples per core = 32
S = 4                      # samples per group
NG = BC // S               # 8 groups per core
RPB = 256                  # rows per block
RJ = 2                     # rows per partition (j dim)
G = S * T // RPB           # 25 blocks per group
AUXW = G * RJ + S * F      # 50 + 320 = 370 floats per group
ROWS = S * T               # 6400 rows per group

FREQ_MASKS = 2
TIME_MASKS = 0.05
FREQ_WIDTH = 27
TIME_WIDTH = 0.05
MAX_TIME_MASKS = 10


def _keep_vectors(length: np.ndarray):
    """Replicate the reference's _spec_masks RNG bit-exactly on host CPU.

    Returns keep_t [B,T] float32 and keep_f [B,F] float32 (1.0 = keep).
    """
    import jax
    import jax.numpy as jnp

    cpu = jax.local_devices(backend="cpu")[0]
    with jax.default_device(cpu):
        length = jnp.asarray(np.asarray(length))
        key = jax.random.key(42)
        kf1, kf2, kt1, kt2 = jax.random.split(key, 4)

        x_left = jax.random.randint(kf1, (B, FREQ_MASKS), 0, F - FREQ_WIDTH + 1)
        w_f = jax.random.randint(kf2, (B, FREQ_MASKS), 0, FREQ_WIDTH + 1)
        f_idx = jnp.arange(F)
        freq_mask = jnp.any(
            (f_idx >= x_left[..., None]) & (f_idx < (x_left + w_f)[..., None]),
            axis=1,
        )  # [B, F]

        lf = length.astype(jnp.float32)
        n_t = jnp.minimum(MAX_TIME_MASKS, (lf * TIME_MASKS).astype(jnp.int32))
        tw = jnp.maximum(1, (lf * TIME_WIDTH).astype(jnp.int32))
        hi = jnp.maximum(1, length - tw)

        u1 = jax.random.uniform(kt1, (B, MAX_TIME_MASKS))
        y_left = jnp.minimum(
            (u1 * (hi[:, None] + 1).astype(jnp.float32)).astype(jnp.int32),
            hi[:, None],
        )
        u2 = jax.random.uniform(kt2, (B, MAX_TIME_MASKS))
        w_t = jnp.minimum(
            (u2 * (tw[:, None] + 1).astype(jnp.float32)).astype(jnp.int32),
            tw[:, None],
        )
        active = jnp.arange(MAX_TIME_MASKS)[None, :] < n_t[:, None]
        t_idx = jnp.arange(T)
        time_mask = jnp.any(
            active[..., None]
            & (t_idx >= y_left[..., None])
            & (t_idx < (y_left + w_t)[..., None]),
            axis=1,
        )  # [B, T]

        keep_t = 1.0 - np.asarray(time_mask, dtype=np.float32)
        keep_f = 1.0 - np.asarray(freq_mask, dtype=np.float32)
    return keep_t, keep_f


def _sample_rects():
    """Per-sample (block-range, partition-range) rectangles within a group.

    Returns a list of (s, ops) where ops is a list of
    (g_lo, g_hi, p_lo, p_hi) rectangles covering sample s's rows.
    """
    out = []
    for s in range(S):
        rs, re = s * T, (s + 1) * T
        bs, be = rs // RPB, (re - 1) // RPB
        ops = []
        first_full = bs
        if rs % RPB:
            ps = (rs % RPB) // RJ
            ops.append((bs, bs + 1, ps, 128))
            first_full = bs + 1
        last_full_excl = be if (re % RPB) else be + 1
        if last_full_excl > first_full:
            ops.append((first_full, last_full_excl, 0, 128))
        if re % RPB:
            pe = (re % RPB) // RJ
            ops.append((be, be + 1, 0, pe))
        # HW: an AP's partition span is limited by its base partition
        # (base 0 -> 128, 32 -> 32, 64 -> 64, 96 -> 32). Split rects.
        legal = []
        for g_lo, g_hi, p_lo, p_hi in ops:
            while p_lo < p_hi:
                span = p_hi - p_lo if p_lo == 0 else min(64 if p_lo == 64 else 32, p_hi - p_lo)
                legal.append((g_lo, g_hi, p_lo, p_lo + span))
                p_lo += span
        out.append((s, legal))
    return out


_NC_CACHE = {}


def _build_bass():
    import concourse.bass as bass
    import concourse.tile as tile
    from concourse import bacc, mybir

    f32 = mybir.dt.float32
    nc = bacc.Bacc()
    x = nc.declare_dram_parameter("x", [BC, T, F], f32, isOutput=False)
    aux = nc.declare_dram_parameter("aux", [128, NG * AUXW], f32, isOutput=False)
    y = nc.declare_dram_parameter("y", [BC, T, F], f32, isOutput=True)

    # [group, block, partition, j, f] view of the contiguous shard
    xv = x[:].flatten_outer_dims().rearrange(
        "(grp g p j) f -> grp g p j f", g=G, p=128, j=RJ
    )
    yv = y[:].flatten_outer_dims().rearrange(
        "(grp g p j) f -> grp g p j f", g=G, p=128, j=RJ
    )
    rects = _sample_rects()

    GH = (G + 1) // 2  # 13: first-half block count

    with tile.TileContext(nc) as tc:
        with (
            tc.tile_pool(name="data", bufs=6) as data,
            tc.tile_pool(name="auxp", bufs=1) as auxp,
        ):
            aux_all = auxp.tile([128, NG * AUXW], f32)
            nc.gpsimd.dma_start(out=aux_all[:], in_=aux[:])
            for grp in range(NG):
                atile = aux_all[:, grp * AUXW : (grp + 1) * AUXW]

                for b_lo, b_hi in ((0, GH), (GH, G)):
                    nb = b_hi - b_lo
                    dtile = data.tile([128, nb * RJ * F], f32, tag="d")
                    d4 = dtile[:].rearrange("p (g j f) -> p g j f", j=RJ, f=F)
                    nc.sync.dma_start(
                        out=d4,
                        in_=xv[grp][b_lo:b_hi].rearrange("g p j f -> p g j f"),
                    )

                    kt = (
                        atile[:, b_lo * RJ : b_hi * RJ]
                        .rearrange("p (g j) -> p g j", j=RJ)
                        .unsqueeze(3)
                        .broadcast_to([128, nb, RJ, F])
                    )
                    nc.vector.tensor_mul(d4, d4, kt)

                    for s, ops in rects:
                        kf = atile[:, G * RJ + s * F : G * RJ + (s + 1) * F]
                        for g_lo, g_hi, p_lo, p_hi in ops:
                            if g_lo >= b_hi or g_hi <= b_lo:
                                continue
                            gl, gh = g_lo - b_lo, g_hi - b_lo
                            dst = d4[p_lo:p_hi, gl:gh, :, :]
                            src = (
                                kf[p_lo:p_hi]
                                .unsqueeze(1)
                                .unsqueeze(2)
                                .broadcast_to([p_hi - p_lo, gh - gl, RJ, F])
                            )
                            nc.vector.tensor_mul(dst, dst, src)

                    nc.scalar.dma_start(
                        out=yv[grp][b_lo:b_hi].rearrange("g p j f -> p g j f"),
                        in_=d4,
                    )
    if not nc.is_finalized():
        nc.finalize()
    return nc


def _get_nc():
    if "nc" not in _NC_CACHE:
        _NC_CACHE["nc"] = _build_bass()
    return _NC_CACHE["nc"]


def _pack_aux(keep_t: np.ndarray, keep_f: np.ndarray) -> np.ndarray:
    # kt table per group: [p, g*RJ + j] = keep_t at group row g*RPB + p*RJ + j
    kt = keep_t.reshape(N_CORES, NG, G, 128, RJ).transpose(0, 1, 3, 2, 4)
    kt = kt.reshape(N_CORES, NG, 128, G * RJ)
    kf = keep_f.reshape(N_CORES, NG, S, F)
    aux = np.empty((N_CORES, NG, 128, AUXW), np.float32)
    aux[..., : G * RJ] = kt
    for s in range(S):
        aux[..., G * RJ + s * F : G * RJ + (s + 1) * F] = kf[:, :, s][:, :, None, :]
    # device layout: one [128, NG*AUXW] block per core
    return np.ascontiguousarray(aux.transpose(0, 2, 1, 3)).reshape(
        N_CORES, 128, NG * AUXW
    )


LAST_RESULTS = None


def kernel(input_spec: np.ndarray, length: np.ndarray):
    global LAST_RESULTS
    from concourse.bass_utils import run_bass_kernel_spmd

    input_spec = np.asarray(input_spec, dtype=np.float32)
    length = np.asarray(length, dtype=np.int32)
    assert input_spec.shape == (B, T, F), input_spec.shape

    keep_t, keep_f = _keep_vectors(length)
    aux = _pack_aux(keep_t, keep_f)

    in_maps = [
        {
            "x": np.ascontiguousarray(input_spec[c * BC : (c + 1) * BC]),
            "aux": aux[c],
        }
        for c in range(N_CORES)
    ]

    nc = _get_nc()
    res = run_bass_kernel_spmd(nc, in_maps, core_ids=list(range(N_CORES)))
    LAST_RESULTS = res
    out = np.concatenate([res.results[c]["y"] for c in range(N_CORES)], axis=0)
    return out, length
